# revision 12
# baseline (speedup 1.0000x reference)
"""Trainium2 Bass kernel for dual-softmax cosine-similarity attention.

Per batch b:
    pn = p / ||p||,  qn = q / ||q||           (L2 over D)
    S  = pn @ qn^T                            [L, L]
    out_p = softmax(S, axis=1) @ q            [L, D]
    out_q = softmax(S, axis=0) @ p            [L, D]

Shapes: B=64, L=512, D=768 fp32. Data-parallel over B across 8 cores
(8 batches per core).

Host prep: p/q are L2-normalized (row norms ~ sqrt(768) >> eps so the
eps guard is moot), scaled by 16 and shipped as fp8e4 in a k-pair-packed
transposed layout for DoubleRow matmuls; raw p/q ship as fp16. All host
arrays are PARTITION-MAJOR (leading 128 dim) so every DMA is one
dma_start with 128 contiguous per-partition descriptors — descriptor
generation on the Sync queue was the hidden bottleneck of the previous
revision. q gets a ones column appended so the out_p matmul also
produces the softmax row-sums (fused rowsum); p is padded to match.

On-chip per batch (layouts avoid any on-chip transposes):
    G^T[j,i] = sum_d (16 qn)^T (16 pn)       fp8 DoubleRow matmuls, PSUM
    E^T      = exp(G^T / 256), colsum[j] fused accum   (ACT, from PSUM)
    F        = E^T * (1/colsum[j])           per-partition scale (DVE)
    out_p[i,:]: PSUM = E^T.T @ [q | 1]; col 256 of the B chunk holds
                rowsum_i; ACT evac scales by 1/rowsum_i -> fp16
    out_q[i,:] = F.T @ p                     (DVE evac -> fp16)
out_p stores issue on the ACT HW-DGE queue, out_q stores on the Sync
queue, so stores never serialize behind input loads.
Softmax max-subtraction is skipped: S entries are cosines in [-1,1].
"""

import numpy as np
import ml_dtypes

B, L, D = 64, 512, 768
N_CORES = 8
BPC = B // N_CORES  # batches per core
LT = L // 128  # 4
DT = D // 128  # 6
DP = DT // 2  # 3 k-pairs for DoubleRow
SCALE = 16.0  # host pre-scale on normalized operands
USE_DR = True  # fp8 DoubleRow perf mode for the similarity matmul
PREWARM = 0  # dummy PE matmuls at start to release the HAM clock gate

_cache = {}


def _build(use_dr=USE_DR, bpc=BPC, prewarm=0):
    import concourse.tile as tile
    import concourse.mybir as mybir
    from concourse import bacc

    f32 = mybir.dt.float32
    f16 = mybir.dt.float16
    f8 = mybir.dt.float8e4
    AF = mybir.ActivationFunctionType
    DR = mybir.MatmulPerfMode.DoubleRow

    nc = bacc.Bacc("TRN2", target_bir_lowering=False, debug=False)

    # [b, p, s(p/q), t, k, n] fp8 — partition-major, per-partition contiguous
    pq_t = nc.dram_tensor(
        "pq_t", [bpc, 128, 2, DP, 2, L], f8, kind="ExternalInput"
    ).ap()
    # [b, p, s(p/q), t, n(769)] fp16 — col 768 is 0 for p, 1 for q
    pq_nat = nc.dram_tensor(
        "pq_nat", [bpc, 128, 2, LT, D + 1], f16, kind="ExternalInput"
    ).ap()
    # [b, p, t, s(p/q), n] fp16 — row l = t*128 + p (host unshuffles)
    out_pq = nc.dram_tensor(
        "out_pq", [bpc, 128, LT, 2, D], f16, kind="ExternalOutput"
    ).ap()

    with tile.TileContext(nc) as tc:
        with (
            tc.tile_pool(name="singles", bufs=1) as singles,
            tc.tile_pool(name="inp", bufs=5) as inp,
            tc.tile_pool(name="ew", bufs=2) as ew,
            tc.tile_pool(name="small", bufs=2) as small,
            tc.tile_pool(name="outs", bufs=4) as outs,
            tc.tile_pool(name="g_ps", bufs=2, space="PSUM") as g_ps,
            tc.tile_pool(name="oa_ps", bufs=3, space="PSUM") as oa_ps,
            tc.tile_pool(name="ob_ps", bufs=2, space="PSUM") as ob_ps,
        ):
            state = {}

            def emit_prewarm(n_mm):
                """Dummy matmuls during the initial DMA wait keep the PE
                busy so the HAM clock gate releases (K=8/8) before the
                first real similarity matmul arrives."""
                warm = singles.tile([128, 512], f8, tag="warm")
                nc.vector.memset(warm, 0.0)
                wp = g_ps.tile([128, 512], f32, tag="g", name="warm_ps")
                for i in range(n_mm):
                    nc.tensor.matmul(
                        wp,
                        lhsT=warm[:, 0:128],
                        rhs=warm,
                        start=(i == 0),
                        stop=(i == n_mm - 1),
                    )

            def emit_load(b):
                pqt = inp.tile([128, 2, DP, 2, L], f8, tag="pqt", name=f"pqt{b}")
                pqn = inp.tile([128, 2, LT, D + 1], f16, tag="pqn", name=f"pqn{b}")
                nc.sync.dma_start(pqt, pq_t[b])
                nc.sync.dma_start(pqn, pq_nat[b])
                state[b] = dict(pqt=pqt, pqn=pqn)

            def emit_g_exp(b):
                st = state[b]
                pqt = st["pqt"]
                et = ew.tile([128, LT, L], f16, tag="et", name=f"et{b}")
                f = ew.tile([128, LT, L], f16, tag="f", name=f"f{b}")
                colsum = small.tile([128, LT], f32, tag="cs", name=f"cs{b}")
                rcol = small.tile([128, LT], f32, tag="rc", name=f"rc{b}")
                for jt in range(LT):
                    gp = g_ps.tile([128, L], f32, tag="g", name=f"g{b}_{jt}")
                    mm = slice(jt * 128, (jt + 1) * 128)
                    if use_dr:
                        for t in range(DP):
                            nc.tensor.matmul(
                                gp,
                                lhsT=pqt[:, 1, t, :, mm],
                                rhs=pqt[:, 0, t, :, :],
                                start=(t == 0),
                                stop=(t == DP - 1),
                                perf_mode=DR,
                            )
                    else:
                        for t in range(DP):
                            for k in range(2):
                                nc.tensor.matmul(
                                    gp,
                                    lhsT=pqt[:, 1, t, k, mm],
                                    rhs=pqt[:, 0, t, k, :],
                                    start=(t == 0 and k == 0),
                                    stop=(t == DP - 1 and k == 1),
                                )
                    nc.scalar.activation(
                        et[:, jt, :],
                        gp,
                        AF.Exp,
                        scale=1.0 / (SCALE * SCALE),
                        accum_out=colsum[:, jt : jt + 1],
                    )
                    nc.vector.reciprocal(rcol[:, jt : jt + 1], colsum[:, jt : jt + 1])
                    nc.vector.tensor_scalar_mul(
                        f[:, jt, :], et[:, jt, :], rcol[:, jt : jt + 1]
                    )
                st["et"] = et
                st["f"] = f

            def emit_out(b):
                st = state[b]
                pqn, et, f = st["pqn"], st["et"], st["f"]
                for m in range(LT):
                    osb = outs.tile([128, 2, D], f16, tag="osb", name=f"osb{b}_{m}")
                    mm = slice(m * 128, (m + 1) * 128)
                    # out_p: weights E^T, rhs [q | 1]; rowsum in col 256 of B
                    pa = oa_ps.tile([128, 512], f32, tag="oa", name=f"pa{b}_{m}")
                    pb = ob_ps.tile([128, 257], f32, tag="ob", name=f"pb{b}_{m}")
                    for jt in range(LT):
                        nc.tensor.matmul(
                            pa,
                            lhsT=et[:, jt, mm],
                            rhs=pqn[:, 1, jt, 0:512],
                            start=(jt == 0),
                            stop=(jt == LT - 1),
                        )
                    for jt in range(LT):
                        nc.tensor.matmul(
                            pb,
                            lhsT=et[:, jt, mm],
                            rhs=pqn[:, 1, jt, 512:769],
                            start=(jt == 0),
                            stop=(jt == LT - 1),
                        )
                    rinv = small.tile([128, 1], f32, tag="rinv", name=f"ri{b}_{m}")
                    nc.vector.reciprocal(rinv, pb[:, 256:257])
                    nc.scalar.activation(osb[:, 0, 0:512], pa, AF.Copy, scale=rinv)
                    nc.scalar.activation(
                        osb[:, 0, 512:768], pb[:, 0:256], AF.Copy, scale=rinv
                    )
                    # out_q: weights F, rhs p
                    qaps = oa_ps.tile([128, 512], f32, tag="oa", name=f"qa{b}_{m}")
                    qbps = ob_ps.tile([128, 257], f32, tag="ob", name=f"qb{b}_{m}")
                    for jt in range(LT):
                        nc.tensor.matmul(
                            qaps,
                            lhsT=f[:, jt, mm],
                            rhs=pqn[:, 0, jt, 0:512],
                            start=(jt == 0),
                            stop=(jt == LT - 1),
                        )
                    for jt in range(LT):
                        nc.tensor.matmul(
                            qbps[:, 0:256],
                            lhsT=f[:, jt, mm],
                            rhs=pqn[:, 0, jt, 512:768],
                            start=(jt == 0),
                            stop=(jt == LT - 1),
                        )
                    nc.vector.tensor_copy(osb[:, 1, 0:512], qaps)
                    nc.vector.tensor_copy(osb[:, 1, 512:768], qbps[:, 0:256])
                    eng = nc.sync if m % 2 == 0 else nc.scalar
                    eng.dma_start(out_pq[b, :, m, :, :], osb)

            # software pipeline: loads run 3 batches ahead; out matmuls for
            # batch b-1 are emitted after batch b's similarity matmuls so
            # the PE never waits on the exp/F chain of the current batch
            emit_load(0)
            emit_load(1)
            emit_load(2)
            emit_load(3)
            if prewarm:
                emit_prewarm(prewarm)
            for b in range(bpc):
                emit_g_exp(b)
                if b > 0:
                    emit_out(b - 1)
                if b + 4 < bpc:
                    emit_load(b + 4)
            emit_out(bpc - 1)

    nc.compile()
    return nc


def _get_nc():
    key = ("v7", USE_DR, PREWARM)
    if key not in _cache:
        _cache[key] = _build(USE_DR, prewarm=PREWARM)
    return _cache[key]


def _prep_t(x):
    """[bpc, L, D] fp32 -> fp8e4 [bpc, 128, DP, 2, L] (normalized, x16,
    partition-major k-pair-packed transpose)."""
    n = np.sqrt((x * x).sum(axis=2, keepdims=True))
    xn = (SCALE / np.maximum(n, 1e-8)) * x
    xt = xn.transpose(0, 2, 1)  # [bpc, D, L]
    return np.ascontiguousarray(
        xt.reshape(BPC, DP, 2, 128, L).transpose(0, 3, 1, 2, 4)
    ).astype(ml_dtypes.float8_e4m3)


def _prep_nat(x, pad_val):
    """[bpc, L, D] fp32 -> fp16 [bpc, 128, LT, D+1] partition-major with
    a constant column appended."""
    pad = np.full((BPC, L, 1), pad_val, np.float16)
    xp = np.concatenate([x.astype(np.float16), pad], axis=2)  # [bpc, L, 769]
    return np.ascontiguousarray(xp.reshape(BPC, LT, 128, D + 1).transpose(0, 2, 1, 3))


def _unshuffle(arr):
    """[bpc, 128, LT, D] -> [bpc, L, D] fp32 (row l = t*128 + p)."""
    return arr.transpose(0, 2, 1, 3).reshape(BPC, L, D).astype(np.float32)


def kernel(p, q):
    from concourse.bass_utils import run_bass_kernel_spmd

    nc = _get_nc()
    p = np.asarray(p, dtype=np.float32)
    q = np.asarray(q, dtype=np.float32)

    in_maps = []
    for c in range(N_CORES):
        sl = slice(c * BPC, (c + 1) * BPC)
        ps, qs = p[sl], q[sl]
        pq_t = np.stack([_prep_t(ps), _prep_t(qs)], axis=2)
        pq_nat = np.stack([_prep_nat(ps, 0.0), _prep_nat(qs, 1.0)], axis=2)
        in_maps.append(
            {
                "pq_t": np.ascontiguousarray(pq_t),
                "pq_nat": np.ascontiguousarray(pq_nat),
            }
        )

    res = run_bass_kernel_spmd(nc, in_maps, core_ids=list(range(N_CORES)))
    _cache["last_result"] = res
    vec_att_p = np.concatenate(
        [_unshuffle(r["out_pq"][:, :, :, 0]) for r in res.results], axis=0
    )
    vec_att_q = np.concatenate(
        [_unshuffle(r["out_pq"][:, :, :, 1]) for r in res.results], axis=0
    )
    return vec_att_p, vec_att_q


if __name__ == "__main__":
    rng = np.random.default_rng(0)
    p = rng.standard_normal((B, L, D)).astype(np.float32)
    q = rng.standard_normal((B, L, D)).astype(np.float32)
    op, oq = kernel(p, q)
    print("shapes:", op.shape, oq.shape, op.dtype, oq.dtype)


# revision 13
# speedup vs baseline: 1.1780x; 1.1780x over previous
"""Trainium2 Bass kernel for dual-softmax cosine-similarity attention.

Per batch b:
    pn = p / ||p||,  qn = q / ||q||           (L2 over D)
    S  = pn @ qn^T                            [L, L]
    out_p = softmax(S, axis=1) @ q            [L, D]
    out_q = softmax(S, axis=0) @ p            [L, D]

Shapes: B=64, L=512, D=768 fp32. Data-parallel over B across 8 cores
(8 batches per core).

Host prep: p/q are L2-normalized (row norms ~ sqrt(768) >> eps so the
eps guard is moot), scaled by 16 and shipped as fp8e4 in a k-pair-packed
transposed layout for DoubleRow matmuls; raw p/q ship as fp16. All host
arrays are PARTITION-MAJOR (leading 128 dim) so every DMA is one
dma_start with 128 contiguous per-partition descriptors — descriptor
generation on the Sync queue was the hidden bottleneck of the previous
revision. q gets a ones column appended so the out_p matmul also
produces the softmax row-sums (fused rowsum); p is padded to match.

On-chip per batch (layouts avoid any on-chip transposes):
    G^T[j,i] = sum_d (16 qn)^T (16 pn)       fp8 DoubleRow matmuls, PSUM
    E^T      = exp(G^T / 256), colsum[j] fused accum   (ACT, from PSUM)
    F        = E^T * (1/colsum[j])           per-partition scale (DVE)
    out_p[i,:]: PSUM = E^T.T @ [q | 1]; col 256 of the B chunk holds
                rowsum_i; ACT evac scales by 1/rowsum_i -> fp16
    out_q[i,:] = F.T @ p                     (DVE evac -> fp16)
out_p stores issue on the ACT HW-DGE queue, out_q stores on the Sync
queue, so stores never serialize behind input loads.
Softmax max-subtraction is skipped: S entries are cosines in [-1,1].
"""

import numpy as np
import ml_dtypes

B, L, D = 64, 512, 768
N_CORES = 8
BPC = B // N_CORES  # batches per core
LT = L // 128  # 4
DT = D // 128  # 6
DP = DT // 2  # 3 k-pairs for DoubleRow
SCALE = 16.0  # host pre-scale on normalized operands
USE_DR = True  # fp8 DoubleRow perf mode for the similarity matmul
PREWARM = 0  # dummy PE matmuls at start to release the HAM clock gate

_cache = {}


def _build(use_dr=USE_DR, bpc=BPC, prewarm=0):
    import concourse.tile as tile
    import concourse.mybir as mybir
    from concourse import bacc

    f32 = mybir.dt.float32
    f16 = mybir.dt.float16
    f8 = mybir.dt.float8e4
    AF = mybir.ActivationFunctionType
    DR = mybir.MatmulPerfMode.DoubleRow

    nc = bacc.Bacc("TRN2", target_bir_lowering=False, debug=False)

    # [b, p, s(p/q), t, k, n] fp8 — partition-major, per-partition contiguous
    pq_t = nc.dram_tensor(
        "pq_t", [bpc, 128, 2, DP, 2, L], f8, kind="ExternalInput"
    ).ap()
    # [b, p, s(p/q), t, n(769)] fp16 — col 768 is 0 for p, 1 for q
    pq_nat = nc.dram_tensor(
        "pq_nat", [bpc, 128, 2, LT, D + 1], f16, kind="ExternalInput"
    ).ap()
    # [b, p, s(p/q), t, n] fp16 — row l = t*128 + p (host unshuffles)
    out_pq = nc.dram_tensor(
        "out_pq", [bpc, 128, 2, LT, D], f16, kind="ExternalOutput"
    ).ap()

    with tile.TileContext(nc) as tc:
        with (
            tc.tile_pool(name="singles", bufs=1) as singles,
            tc.tile_pool(name="inp", bufs=4) as inp,
            tc.tile_pool(name="ew", bufs=2) as ew,
            tc.tile_pool(name="small", bufs=2) as small,
            tc.tile_pool(name="outs", bufs=4) as outs,
            tc.tile_pool(name="g_ps", bufs=2, space="PSUM") as g_ps,
            tc.tile_pool(name="oa_ps", bufs=3, space="PSUM") as oa_ps,
            tc.tile_pool(name="ob_ps", bufs=2, space="PSUM") as ob_ps,
        ):
            state = {}

            def emit_prewarm(n_mm):
                """Dummy matmuls during the initial DMA wait keep the PE
                busy so the HAM clock gate releases (K=8/8) before the
                first real similarity matmul arrives."""
                warm = singles.tile([128, 512], f8, tag="warm")
                nc.vector.memset(warm, 0.0)
                wp = g_ps.tile([128, 512], f32, tag="g", name="warm_ps")
                for i in range(n_mm):
                    nc.tensor.matmul(
                        wp,
                        lhsT=warm[:, 0:128],
                        rhs=warm,
                        start=(i == 0),
                        stop=(i == n_mm - 1),
                    )

            def emit_load(b):
                pqt = inp.tile([128, 2, DP, 2, L], f8, tag="pqt", name=f"pqt{b}")
                pqn = inp.tile([128, 2, LT, D + 1], f16, tag="pqn", name=f"pqn{b}")
                nc.sync.dma_start(pqt, pq_t[b])
                nc.sync.dma_start(pqn, pq_nat[b])
                state[b] = dict(pqt=pqt, pqn=pqn)

            def emit_g_exp(b):
                st = state[b]
                pqt = st["pqt"]
                et = ew.tile([128, LT, L], f16, tag="et", name=f"et{b}")
                f = ew.tile([128, LT, L], f16, tag="f", name=f"f{b}")
                colsum = small.tile([128, LT], f32, tag="cs", name=f"cs{b}")
                rcol = small.tile([128, LT], f32, tag="rc", name=f"rc{b}")
                for jt in range(LT):
                    gp = g_ps.tile([128, L], f32, tag="g", name=f"g{b}_{jt}")
                    mm = slice(jt * 128, (jt + 1) * 128)
                    if use_dr:
                        for t in range(DP):
                            nc.tensor.matmul(
                                gp,
                                lhsT=pqt[:, 1, t, :, mm],
                                rhs=pqt[:, 0, t, :, :],
                                start=(t == 0),
                                stop=(t == DP - 1),
                                perf_mode=DR,
                            )
                    else:
                        for t in range(DP):
                            for k in range(2):
                                nc.tensor.matmul(
                                    gp,
                                    lhsT=pqt[:, 1, t, k, mm],
                                    rhs=pqt[:, 0, t, k, :],
                                    start=(t == 0 and k == 0),
                                    stop=(t == DP - 1 and k == 1),
                                )
                    nc.scalar.activation(
                        et[:, jt, :],
                        gp,
                        AF.Exp,
                        scale=1.0 / (SCALE * SCALE),
                        accum_out=colsum[:, jt : jt + 1],
                    )
                    nc.vector.reciprocal(rcol[:, jt : jt + 1], colsum[:, jt : jt + 1])
                    nc.vector.tensor_scalar_mul(
                        f[:, jt, :], et[:, jt, :], rcol[:, jt : jt + 1]
                    )
                st["et"] = et
                st["f"] = f

            def emit_out(b, last=False):
                st = state[b]
                pqn, et, f = st["pqn"], st["et"], st["f"]
                osb = None
                for m in range(LT):
                    if m % 2 == 0:
                        osb = outs.tile(
                            [128, 2, 2, D], f16, tag="osb", name=f"osb{b}_{m//2}"
                        )
                    mh = m % 2
                    mm = slice(m * 128, (m + 1) * 128)
                    # out_p: weights E^T, rhs [q | 1]; rowsum in col 256 of B
                    pa = oa_ps.tile([128, 512], f32, tag="oa", name=f"pa{b}_{m}")
                    pb = ob_ps.tile([128, 257], f32, tag="ob", name=f"pb{b}_{m}")
                    for jt in range(LT):
                        nc.tensor.matmul(
                            pa,
                            lhsT=et[:, jt, mm],
                            rhs=pqn[:, 1, jt, 0:512],
                            start=(jt == 0),
                            stop=(jt == LT - 1),
                        )
                    for jt in range(LT):
                        nc.tensor.matmul(
                            pb,
                            lhsT=et[:, jt, mm],
                            rhs=pqn[:, 1, jt, 512:769],
                            start=(jt == 0),
                            stop=(jt == LT - 1),
                        )
                    rinv = small.tile([128, 1], f32, tag="rinv", name=f"ri{b}_{m}")
                    nc.vector.reciprocal(rinv, pb[:, 256:257])
                    nc.scalar.activation(osb[:, 0, mh, 0:512], pa, AF.Copy, scale=rinv)
                    nc.scalar.activation(
                        osb[:, 0, mh, 512:768], pb[:, 0:256], AF.Copy, scale=rinv
                    )
                    # out_q: weights F, rhs p
                    qaps = oa_ps.tile([128, 512], f32, tag="oa", name=f"qa{b}_{m}")
                    qbps = ob_ps.tile([128, 257], f32, tag="ob", name=f"qb{b}_{m}")
                    for jt in range(LT):
                        nc.tensor.matmul(
                            qaps,
                            lhsT=f[:, jt, mm],
                            rhs=pqn[:, 0, jt, 0:512],
                            start=(jt == 0),
                            stop=(jt == LT - 1),
                        )
                    for jt in range(LT):
                        nc.tensor.matmul(
                            qbps[:, 0:256],
                            lhsT=f[:, jt, mm],
                            rhs=pqn[:, 0, jt, 512:768],
                            start=(jt == 0),
                            stop=(jt == LT - 1),
                        )
                    nc.vector.tensor_copy(osb[:, 1, mh, 0:512], qaps)
                    nc.vector.tensor_copy(osb[:, 1, mh, 512:768], qbps[:, 0:256])
                    if last:
                        # final batch: drain per m so the tail overlaps evacs
                        nc.sync.dma_start(out_pq[b, :, :, m, :], osb[:, :, mh, :])
                    elif m % 2 == 1:
                        nc.sync.dma_start(out_pq[b, :, :, m - 1 : m + 1, :], osb)

            # software pipeline: loads run 3 batches ahead; out matmuls for
            # batch b-1 are emitted after batch b's similarity matmuls so
            # the PE never waits on the exp/F chain of the current batch
            emit_load(0)
            emit_load(1)
            emit_load(2)
            if prewarm:
                emit_prewarm(prewarm)
            for b in range(bpc):
                emit_g_exp(b)
                if b > 0:
                    emit_out(b - 1)
                if b + 3 < bpc:
                    emit_load(b + 3)
            emit_out(bpc - 1, last=True)

    nc.compile()
    return nc


def _get_nc():
    key = ("v8", USE_DR, PREWARM)
    if key not in _cache:
        _cache[key] = _build(USE_DR, prewarm=PREWARM)
    return _cache[key]


def _prep_t(x):
    """[bpc, L, D] fp32 -> fp8e4 [bpc, 128, DP, 2, L] (normalized, x16,
    partition-major k-pair-packed transpose)."""
    n = np.sqrt((x * x).sum(axis=2, keepdims=True))
    xn = (SCALE / np.maximum(n, 1e-8)) * x
    xt = xn.transpose(0, 2, 1)  # [bpc, D, L]
    return np.ascontiguousarray(
        xt.reshape(BPC, DP, 2, 128, L).transpose(0, 3, 1, 2, 4)
    ).astype(ml_dtypes.float8_e4m3)


def _prep_nat(x, pad_val):
    """[bpc, L, D] fp32 -> fp16 [bpc, 128, LT, D+1] partition-major with
    a constant column appended."""
    pad = np.full((BPC, L, 1), pad_val, np.float16)
    xp = np.concatenate([x.astype(np.float16), pad], axis=2)  # [bpc, L, 769]
    return np.ascontiguousarray(xp.reshape(BPC, LT, 128, D + 1).transpose(0, 2, 1, 3))


def _unshuffle(arr):
    """[bpc, 128, LT, D] -> [bpc, L, D] fp32 (row l = t*128 + p)."""
    return arr.transpose(0, 2, 1, 3).reshape(BPC, L, D).astype(np.float32)


def kernel(p, q):
    from concourse.bass_utils import run_bass_kernel_spmd

    nc = _get_nc()
    p = np.asarray(p, dtype=np.float32)
    q = np.asarray(q, dtype=np.float32)

    in_maps = []
    for c in range(N_CORES):
        sl = slice(c * BPC, (c + 1) * BPC)
        ps, qs = p[sl], q[sl]
        pq_t = np.stack([_prep_t(ps), _prep_t(qs)], axis=2)
        pq_nat = np.stack([_prep_nat(ps, 0.0), _prep_nat(qs, 1.0)], axis=2)
        in_maps.append(
            {
                "pq_t": np.ascontiguousarray(pq_t),
                "pq_nat": np.ascontiguousarray(pq_nat),
            }
        )

    res = run_bass_kernel_spmd(nc, in_maps, core_ids=list(range(N_CORES)))
    _cache["last_result"] = res
    vec_att_p = np.concatenate(
        [_unshuffle(r["out_pq"][:, :, 0]) for r in res.results], axis=0
    )
    vec_att_q = np.concatenate(
        [_unshuffle(r["out_pq"][:, :, 1]) for r in res.results], axis=0
    )
    return vec_att_p, vec_att_q


if __name__ == "__main__":
    rng = np.random.default_rng(0)
    p = rng.standard_normal((B, L, D)).astype(np.float32)
    q = rng.standard_normal((B, L, D)).astype(np.float32)
    op, oq = kernel(p, q)
    print("shapes:", op.shape, oq.shape, op.dtype, oq.dtype)


# revision 14
# speedup vs baseline: 1.1825x; 1.0039x over previous
"""Trainium2 Bass kernel for dual-softmax cosine-similarity attention.

Per batch b:
    pn = p / ||p||,  qn = q / ||q||           (L2 over D)
    S  = pn @ qn^T                            [L, L]
    out_p = softmax(S, axis=1) @ q            [L, D]
    out_q = softmax(S, axis=0) @ p            [L, D]

Shapes: B=64, L=512, D=768 fp32. Data-parallel over B across 8 cores
(8 batches per core).

Host prep: p/q are L2-normalized (row norms ~ sqrt(768) >> eps so the
eps guard is moot), scaled by 16 and shipped as fp8e4 in a k-pair-packed
transposed layout for DoubleRow matmuls; raw p/q ship as fp16. All host
arrays are PARTITION-MAJOR (leading 128 dim) so every DMA is one
dma_start with 128 contiguous per-partition descriptors — descriptor
generation on the Sync queue was the hidden bottleneck of the previous
revision. q gets a ones column appended so the out_p matmul also
produces the softmax row-sums (fused rowsum); p is padded to match.

On-chip per batch (layouts avoid any on-chip transposes):
    G^T[j,i] = sum_d (16 qn)^T (16 pn)       fp8 DoubleRow matmuls, PSUM
    E^T      = exp(G^T / 256), colsum[j] fused accum   (ACT, from PSUM)
    F        = E^T * (1/colsum[j])           per-partition scale (DVE)
    out_p[i,:]: PSUM = E^T.T @ [q | 1]; col 256 of the B chunk holds
                rowsum_i; ACT evac scales by 1/rowsum_i -> fp16
    out_q[i,:] = F.T @ p                     (DVE evac -> fp16)
out_p stores issue on the ACT HW-DGE queue, out_q stores on the Sync
queue, so stores never serialize behind input loads.
Softmax max-subtraction is skipped: S entries are cosines in [-1,1].
"""

import numpy as np
import ml_dtypes

B, L, D = 64, 512, 768
N_CORES = 8
BPC = B // N_CORES  # batches per core
LT = L // 128  # 4
DT = D // 128  # 6
DP = DT // 2  # 3 k-pairs for DoubleRow
SCALE = 16.0  # host pre-scale on normalized operands
USE_DR = True  # fp8 DoubleRow perf mode for the similarity matmul
PREWARM = 0  # dummy PE matmuls at start to release the HAM clock gate

_cache = {}


def _build(use_dr=USE_DR, bpc=BPC, prewarm=0):
    import concourse.tile as tile
    import concourse.mybir as mybir
    from concourse import bacc

    f32 = mybir.dt.float32
    f16 = mybir.dt.float16
    f8 = mybir.dt.float8e4
    AF = mybir.ActivationFunctionType
    DR = mybir.MatmulPerfMode.DoubleRow

    nc = bacc.Bacc("TRN2", target_bir_lowering=False, debug=False)

    # [b, p, s(p/q), t, k, n] fp8 — partition-major, per-partition contiguous
    pq_t = nc.dram_tensor(
        "pq_t", [bpc, 128, 2, DP, 2, L], f8, kind="ExternalInput"
    ).ap()
    # [b, p, s(p/q), t, n(769)] fp16 — col 768 is 0 for p, 1 for q
    pq_nat = nc.dram_tensor(
        "pq_nat", [bpc, 128, 2, LT, D + 1], f16, kind="ExternalInput"
    ).ap()
    # [b, p, s(p/q), t, n] fp16 — row l = t*128 + p (host unshuffles)
    out_pq = nc.dram_tensor(
        "out_pq", [bpc, 128, 2, LT, D], f16, kind="ExternalOutput"
    ).ap()

    with tile.TileContext(nc) as tc:
        with (
            tc.tile_pool(name="singles", bufs=1) as singles,
            tc.tile_pool(name="inp", bufs=4) as inp,
            tc.tile_pool(name="ew", bufs=2) as ew,
            tc.tile_pool(name="small", bufs=2) as small,
            tc.tile_pool(name="outs", bufs=4) as outs,
            tc.tile_pool(name="g_ps", bufs=2, space="PSUM") as g_ps,
            tc.tile_pool(name="oa_ps", bufs=3, space="PSUM") as oa_ps,
            tc.tile_pool(name="ob_ps", bufs=2, space="PSUM") as ob_ps,
        ):
            state = {}

            def emit_prewarm(n_mm):
                """Dummy matmuls during the initial DMA wait keep the PE
                busy so the HAM clock gate releases (K=8/8) before the
                first real similarity matmul arrives."""
                warm = singles.tile([128, 512], f8, tag="warm")
                nc.vector.memset(warm, 0.0)
                wp = g_ps.tile([128, 512], f32, tag="g", name="warm_ps")
                for i in range(n_mm):
                    nc.tensor.matmul(
                        wp,
                        lhsT=warm[:, 0:128],
                        rhs=warm,
                        start=(i == 0),
                        stop=(i == n_mm - 1),
                    )

            def emit_load(b):
                pqt = inp.tile([128, 2, DP, 2, L], f8, tag="pqt", name=f"pqt{b}")
                pqn = inp.tile([128, 2, LT, D + 1], f16, tag="pqn", name=f"pqn{b}")
                nc.sync.dma_start(pqt, pq_t[b])
                nc.sync.dma_start(pqn, pq_nat[b])
                state[b] = dict(pqt=pqt, pqn=pqn)

            def emit_g_exp(b):
                st = state[b]
                pqt = st["pqt"]
                et = ew.tile([128, LT, L], f16, tag="et", name=f"et{b}")
                f = ew.tile([128, LT, L], f16, tag="f", name=f"f{b}")
                colsum = small.tile([128, LT], f32, tag="cs", name=f"cs{b}")
                rcol = small.tile([128, LT], f32, tag="rc", name=f"rc{b}")
                for jt in range(LT):
                    gp = g_ps.tile([128, L], f32, tag="g", name=f"g{b}_{jt}")
                    mm = slice(jt * 128, (jt + 1) * 128)
                    if use_dr:
                        for t in range(DP):
                            nc.tensor.matmul(
                                gp,
                                lhsT=pqt[:, 1, t, :, mm],
                                rhs=pqt[:, 0, t, :, :],
                                start=(t == 0),
                                stop=(t == DP - 1),
                                perf_mode=DR,
                            )
                    else:
                        for t in range(DP):
                            for k in range(2):
                                nc.tensor.matmul(
                                    gp,
                                    lhsT=pqt[:, 1, t, k, mm],
                                    rhs=pqt[:, 0, t, k, :],
                                    start=(t == 0 and k == 0),
                                    stop=(t == DP - 1 and k == 1),
                                )
                    nc.scalar.activation(
                        et[:, jt, :],
                        gp,
                        AF.Exp,
                        scale=1.0 / (SCALE * SCALE),
                        accum_out=colsum[:, jt : jt + 1],
                    )
                    nc.vector.reciprocal(rcol[:, jt : jt + 1], colsum[:, jt : jt + 1])
                    nc.vector.tensor_scalar_mul(
                        f[:, jt, :], et[:, jt, :], rcol[:, jt : jt + 1]
                    )
                st["et"] = et
                st["f"] = f

            def emit_out(b, last=False):
                st = state[b]
                pqn, et, f = st["pqn"], st["et"], st["f"]
                osb = None
                for m in range(LT):
                    if m % 2 == 0:
                        osb = outs.tile(
                            [128, 2, 2, D], f16, tag="osb", name=f"osb{b}_{m//2}"
                        )
                    mh = m % 2
                    mm = slice(m * 128, (m + 1) * 128)
                    # out_p: weights E^T, rhs [q | 1]; rowsum in col 256 of B
                    pa = oa_ps.tile([128, 512], f32, tag="oa", name=f"pa{b}_{m}")
                    pb = ob_ps.tile([128, 257], f32, tag="ob", name=f"pb{b}_{m}")
                    for jt in range(LT):
                        nc.tensor.matmul(
                            pa,
                            lhsT=et[:, jt, mm],
                            rhs=pqn[:, 1, jt, 0:512],
                            start=(jt == 0),
                            stop=(jt == LT - 1),
                        )
                    for jt in range(LT):
                        nc.tensor.matmul(
                            pb,
                            lhsT=et[:, jt, mm],
                            rhs=pqn[:, 1, jt, 512:769],
                            start=(jt == 0),
                            stop=(jt == LT - 1),
                        )
                    rinv = small.tile([128, 1], f32, tag="rinv", name=f"ri{b}_{m}")
                    nc.vector.reciprocal(rinv, pb[:, 256:257])
                    nc.scalar.activation(osb[:, 0, mh, 0:512], pa, AF.Copy, scale=rinv)
                    nc.scalar.activation(
                        osb[:, 0, mh, 512:768], pb[:, 0:256], AF.Copy, scale=rinv
                    )
                    # out_q: weights F, rhs p
                    qaps = oa_ps.tile([128, 512], f32, tag="oa", name=f"qa{b}_{m}")
                    qbps = ob_ps.tile([128, 257], f32, tag="ob", name=f"qb{b}_{m}")
                    for jt in range(LT):
                        nc.tensor.matmul(
                            qaps,
                            lhsT=f[:, jt, mm],
                            rhs=pqn[:, 0, jt, 0:512],
                            start=(jt == 0),
                            stop=(jt == LT - 1),
                        )
                    for jt in range(LT):
                        nc.tensor.matmul(
                            qbps[:, 0:256],
                            lhsT=f[:, jt, mm],
                            rhs=pqn[:, 0, jt, 512:768],
                            start=(jt == 0),
                            stop=(jt == LT - 1),
                        )
                    nc.vector.tensor_copy(osb[:, 1, mh, 0:512], qaps)
                    nc.vector.tensor_copy(osb[:, 1, mh, 512:768], qbps[:, 0:256])
                    if last:
                        # final batch: drain per m on both HW-DGE queues so
                        # the tail overlaps evacs (ACT is idle by now)
                        eng = nc.sync if m % 2 == 0 else nc.scalar
                        eng.dma_start(out_pq[b, :, :, m, :], osb[:, :, mh, :])
                    elif m % 2 == 1:
                        nc.sync.dma_start(out_pq[b, :, :, m - 1 : m + 1, :], osb)

            # software pipeline: loads run 3 batches ahead; out matmuls for
            # batch b-1 are emitted after batch b's similarity matmuls so
            # the PE never waits on the exp/F chain of the current batch
            emit_load(0)
            emit_load(1)
            emit_load(2)
            if prewarm:
                emit_prewarm(prewarm)
            for b in range(bpc):
                emit_g_exp(b)
                if b > 0:
                    emit_out(b - 1)
                if b + 3 < bpc:
                    emit_load(b + 3)
            emit_out(bpc - 1, last=True)

    nc.compile()
    return nc


def _get_nc():
    key = ("v9", USE_DR, PREWARM)
    if key not in _cache:
        _cache[key] = _build(USE_DR, prewarm=PREWARM)
    return _cache[key]


def _prep_t(x):
    """[bpc, L, D] fp32 -> fp8e4 [bpc, 128, DP, 2, L] (normalized, x16,
    partition-major k-pair-packed transpose)."""
    n = np.sqrt((x * x).sum(axis=2, keepdims=True))
    xn = (SCALE / np.maximum(n, 1e-8)) * x
    xt = xn.transpose(0, 2, 1)  # [bpc, D, L]
    return np.ascontiguousarray(
        xt.reshape(BPC, DP, 2, 128, L).transpose(0, 3, 1, 2, 4)
    ).astype(ml_dtypes.float8_e4m3)


def _prep_nat(x, pad_val):
    """[bpc, L, D] fp32 -> fp16 [bpc, 128, LT, D+1] partition-major with
    a constant column appended."""
    pad = np.full((BPC, L, 1), pad_val, np.float16)
    xp = np.concatenate([x.astype(np.float16), pad], axis=2)  # [bpc, L, 769]
    return np.ascontiguousarray(xp.reshape(BPC, LT, 128, D + 1).transpose(0, 2, 1, 3))


def _unshuffle(arr):
    """[bpc, 128, LT, D] -> [bpc, L, D] fp32 (row l = t*128 + p)."""
    return arr.transpose(0, 2, 1, 3).reshape(BPC, L, D).astype(np.float32)


def kernel(p, q):
    from concourse.bass_utils import run_bass_kernel_spmd

    nc = _get_nc()
    p = np.asarray(p, dtype=np.float32)
    q = np.asarray(q, dtype=np.float32)

    in_maps = []
    for c in range(N_CORES):
        sl = slice(c * BPC, (c + 1) * BPC)
        ps, qs = p[sl], q[sl]
        pq_t = np.stack([_prep_t(ps), _prep_t(qs)], axis=2)
        pq_nat = np.stack([_prep_nat(ps, 0.0), _prep_nat(qs, 1.0)], axis=2)
        in_maps.append(
            {
                "pq_t": np.ascontiguousarray(pq_t),
                "pq_nat": np.ascontiguousarray(pq_nat),
            }
        )

    res = run_bass_kernel_spmd(nc, in_maps, core_ids=list(range(N_CORES)))
    _cache["last_result"] = res
    vec_att_p = np.concatenate(
        [_unshuffle(r["out_pq"][:, :, 0]) for r in res.results], axis=0
    )
    vec_att_q = np.concatenate(
        [_unshuffle(r["out_pq"][:, :, 1]) for r in res.results], axis=0
    )
    return vec_att_p, vec_att_q


if __name__ == "__main__":
    rng = np.random.default_rng(0)
    p = rng.standard_normal((B, L, D)).astype(np.float32)
    q = rng.standard_normal((B, L, D)).astype(np.float32)
    op, oq = kernel(p, q)
    print("shapes:", op.shape, oq.shape, op.dtype, oq.dtype)
